# revision 1
# baseline (speedup 1.0000x reference)
"""Trainium2 Bass kernel for CURLoRA forward: out = x @ (C @ U @ R).T

Fused low-rank chain per core (never materializes the [8192, 8192] W):
  t1.T = sum_k R_k.T.T @ x_k.T     (64 K-tiles of 128, f32r, PSUM-accumulated)
  t2.T = [U.T|U.T].T @ t1.T        (one f32r matmul, M=128 duplicates t2.T
                                    into both partition halves)
  out  = t2.T.T @ C.T              (bf16 single-pass: C arrives bf16 via
                                    SWDGE inline DMA cast, t2 copied
                                    psum->bf16; K=64 so rel err ~2e-3)

Sharding (8 cores, no collectives): the 128 rows of x are split 4 ways and
the 8192 output columns 2 ways. Per core DMA: 1MB x-shard + 2MB R
(replicated; irreducible without cross-core comms) + 1MB C.T shard + 0.5MB
out = 4.5MB vs 6.75MB for the "shard C rows only" layout. All transposes
are host-side layout prep during sharding; every FLOP runs on-device.

Hand-scheduled raw bass (no Tile): per-DMA semaphores, engine-parallel
descriptor generation (x on sync, R on scalar, C.T/U on gpsimd), C.T gated
behind the x/R stream so the stage-1-critical bytes keep the HBM bandwidth,
and a pipelined matmul->copy->DMA output tail per PSUM bank. The final
out-DMA completion waits are dropped (incs kept for codegen): the write
receipts drain under the fixed ~8us semaphore-cleanup postamble.
"""

import numpy as np

B, S, M, N, RANK = 2, 64, 8192, 8192, 64
NCORES = 8
SA, NB = 4, 2              # s-blocks x n-blocks = 8 cores
SSH = (B * S) // SA        # 32 s-rows per core
NSH = N // NB              # 4096 out cols per core
KCH = M // 128             # 64 contraction chunks of 128

# k-chunks per x/R DMA piece: small first piece starts the PE early, small
# last piece keeps the post-stream PE chase short
PIECES = (8, 24, 24, 8)

_NC_CACHE = {}


def _build_nc():
    if "nc" in _NC_CACHE:
        return _NC_CACHE["nc"]
    from contextlib import ExitStack
    from concourse import mybir
    import concourse.bass as bass

    f32 = mybir.dt.float32
    f32r = mybir.dt.float32r
    bf16 = mybir.dt.bfloat16
    nc = bass.Bass()

    xp_d = nc.declare_dram_parameter("xp", [128, KCH * SSH], f32r, isOutput=False)
    rp_d = nc.declare_dram_parameter("rp", [128, KCH * RANK], f32r, isOutput=False)
    uq_d = nc.declare_dram_parameter("uq", [RANK, 128], f32r, isOutput=False)
    ct_d = nc.declare_dram_parameter("ct", [128, NSH // 2], f32, isOutput=False)
    out_d = nc.declare_dram_parameter("out", [128, NSH // 4], f32, isOutput=True)

    ctx = ExitStack()
    with ctx:
        xts = [
            ctx.enter_context(nc.sbuf_tensor(f"xt{i}", [128, kw * SSH], f32r))
            for i, kw in enumerate(PIECES)
        ]
        rts = [
            ctx.enter_context(nc.sbuf_tensor(f"rt{i}", [128, kw * RANK], f32r))
            for i, kw in enumerate(PIECES)
        ]
        uqt = ctx.enter_context(nc.sbuf_tensor("uqt", [RANK, 128], f32r))
        # bf16: the gpsimd (SWDGE) DMA casts f32->bf16 inline during the
        # transfer -- full f32 HBM read, half the SBUF writes, no DVE work
        cts = [
            ctx.enter_context(nc.sbuf_tensor(f"ct{i}", [128, 1024], bf16))
            for i in range(2)
        ]
        t1s = ctx.enter_context(nc.sbuf_tensor("t1s", [RANK, SSH], f32r))
        t2b = ctx.enter_context(nc.sbuf_tensor("t2b", [128, SSH], bf16))
        osbs = [
            ctx.enter_context(nc.sbuf_tensor(f"osb{i}", [128, 256], f32))
            for i in range(4)
        ]
        # one PSUM bank each ([128, 512] f32 = exactly one bank)
        ps1 = ctx.enter_context(nc.psum_tensor("ps1", [128, 512], f32))
        ps2 = ctx.enter_context(nc.psum_tensor("ps2", [128, 512], f32))
        psos = [
            ctx.enter_context(nc.psum_tensor(f"pso{i}", [128, 512], f32))
            for i in range(4)
        ]

        # one semaphore per DMA: queue completions of distinct DMAs are not
        # ordered, so a shared counter would be unsound
        sxs = [ctx.enter_context(nc.semaphore(f"sx{i}")) for i in range(len(PIECES))]
        srs = [ctx.enter_context(nc.semaphore(f"sr{i}")) for i in range(len(PIECES))]
        scu = ctx.enter_context(nc.semaphore("scu"))
        scs = [ctx.enter_context(nc.semaphore(f"sc{i}")) for i in range(2)]
        sm = ctx.enter_context(nc.semaphore("sm"))
        sv = ctx.enter_context(nc.semaphore("sv"))
        sos = [ctx.enter_context(nc.semaphore(f"so{i}")) for i in range(4)]

        block = ctx.enter_context(nc.Block())

        @block.sync
        def _(sync):
            off = 0
            for p, kw in enumerate(PIECES):
                sync.dma_start(
                    xts[p][:], xp_d[:, off * SSH:(off + kw) * SSH]
                ).then_inc(sxs[p], 16)
                off += kw
            for cb in (0, 1):
                sync.wait_ge(sv, 3 + cb)
                sync.dma_start(
                    out_d[:, cb * 256:(cb + 1) * 256], osbs[cb][:]
                ).then_inc(sos[cb], 16)
            # wait only on the LAST out DMA of this ring: per-engine FIFO
            # descriptor drain makes its sem imply the earlier DMA finished,
            # and halting engines with DMAs in flight risks wedging the
            # device (NRT_EXEC_UNIT_UNRECOVERABLE observed twice without it)
            sync.wait_ge(sos[1], 16)

        @block.scalar
        def _(scalar):
            off = 0
            for p, kw in enumerate(PIECES):
                scalar.dma_start(
                    rts[p][:], rp_d[:, off * RANK:(off + kw) * RANK]
                ).then_inc(srs[p], 16)
                off += kw
            # second half of the output on the ACT ring: each HWDGE issue
            # occupies its engine ~0.65us, and with no final waits the LAST
            # issue sets body end -- two engines halve the serialization
            for cb in (2, 3):
                scalar.wait_ge(sv, 3 + cb)
                scalar.dma_start(
                    out_d[:, cb * 256:(cb + 1) * 256], osbs[cb][:]
                ).then_inc(sos[cb], 16)
            scalar.wait_ge(sos[3], 16)

        @block.gpsimd
        def _(g):
            g.dma_start(uqt[:], uq_d[:]).then_inc(scu, 16)
            g.wait_ge(sxs[1], 16)  # let the x/R stream lead on HBM bw
            g.dma_start(cts[0][:], ct_d[:, 0:1024]).then_inc(scs[0], 16)
            g.dma_start(cts[1][:], ct_d[:, 1024:2048]).then_inc(scs[1], 16)

        @block.tensor
        def _(t):
            k = 0
            last_mm = None
            for p, kw in enumerate(PIECES):
                t.wait_ge(sxs[p], 16)
                t.wait_ge(srs[p], 16)
                for kl in range(kw):
                    last_mm = nc.tensor.matmul(
                        ps1[0:RANK, 0:SSH],
                        rts[p][:, kl * RANK:(kl + 1) * RANK],
                        xts[p][:, kl * SSH:(kl + 1) * SSH],
                        start=(k == 0), stop=(k == KCH - 1),
                    )
                    k += 1
            last_mm.then_inc(sm, 1)                      # sm=1: stage 1 done
            t.wait_ge(sv, 1)                             # t1s copied
            t.wait_ge(scu, 16)                           # uqt loaded
            nc.tensor.matmul(ps2[:, 0:SSH], uqt[:], t1s[:],
                             start=True, stop=True).then_inc(sm, 1)  # sm=2
            t.wait_ge(sv, 2)                             # t2b copied
            t.wait_ge(scs[0], 16)                        # ct loaded (bf16)
            t.wait_ge(scs[1], 16)
            for cb in range(4):                          # 256-col out block
                rh, hb = cb // 2, cb % 2
                last_mm = None
                for p in range(2):
                    for w in range(2):
                        q = p * 2 + w                    # psum partition quarter
                        c0 = w * 512 + hb * 256
                        last_mm = nc.tensor.matmul(
                            psos[cb][q * SSH:(q + 1) * SSH, 0:256],
                            t2b[rh * 64:(rh + 1) * 64, :],
                            cts[p][rh * 64:(rh + 1) * 64, c0:c0 + 256],
                            start=True, stop=True,
                            tile_position=(rh * 64, q * SSH),
                        )
                last_mm.then_inc(sm, 1)                  # sm=3..6

        @block.vector
        def _(v):
            v.wait_ge(sm, 1)
            nc.vector.tensor_copy(t1s[:], ps1[0:RANK, 0:SSH]).then_inc(sv, 1)
            v.wait_ge(sm, 2)
            nc.vector.tensor_copy(t2b[:], ps2[:, 0:SSH]).then_inc(sv, 1)
            for cb in range(4):
                v.wait_ge(sm, 3 + cb)
                nc.vector.tensor_copy(
                    osbs[cb][:], psos[cb][:, 0:256]
                ).then_inc(sv, 1)

    _NC_CACHE["nc"] = nc
    return nc


def _shard_inputs(x, C, U, R):
    xf = np.asarray(x, np.float32).reshape(B * S, M)
    C = np.asarray(C, np.float32)
    U = np.asarray(U, np.float32)
    R = np.asarray(R, np.float32)

    # rp[p, k*64+r] = R[r, 128k+p]
    rp = np.ascontiguousarray(
        R.reshape(RANK, KCH, 128).transpose(2, 1, 0)
    ).reshape(128, KCH * RANK)
    # uq = U.T duplicated along columns: stage 2's lhsT, M=128 so t2.T lands
    # duplicated in both partition halves (stage 3 reads them as row halves)
    uq = np.ascontiguousarray(np.concatenate([U.T, U.T], axis=1))

    in_maps = []
    for c in range(NCORES):
        i, j = divmod(c, NB)
        xs = xf[i * SSH:(i + 1) * SSH, :]
        # xp[p, k*32+s] = xs[s, 128k+p]
        xp = np.ascontiguousarray(
            xs.reshape(SSH, KCH, 128).transpose(2, 1, 0)
        ).reshape(128, KCH * SSH)
        # ct rows 0:64 = C.T cols [0,2048) of this n-shard, rows 64:128 =
        # cols [2048,4096) -- full 128-partition (= full-bandwidth) DMA
        cT = C[j * NSH:(j + 1) * NSH, :].T  # [64, 4096]
        ct = np.ascontiguousarray(
            np.concatenate([cT[:, :2048], cT[:, 2048:]], axis=0)
        )  # [128, 2048]
        in_maps.append({"xp": xp, "rp": rp, "uq": uq, "ct": ct})
    return in_maps


def _unshard_output(core_outs):
    full = np.empty((B * S, N), np.float32)
    for c in range(NCORES):
        i, j = divmod(c, NB)
        q = core_outs[c]  # [128, 1024]: q[32a+s, 512h+nr] = out[s, (4h+a)*512+nr]
        blk = q.reshape(4, SSH, 2, 512).transpose(1, 2, 0, 3).reshape(SSH, NSH)
        full[i * SSH:(i + 1) * SSH, j * NSH:(j + 1) * NSH] = blk
    return full.reshape(B, S, N)


def _ensure_ntff_hook():
    """bass_utils' axon trace path imports antenv.axon_hooks, which this
    container's antenv lacks. Register an equivalent module backed by the
    boot package's ctypes NTFF hook so trace=True (or BASS_TRACE=1) works."""
    import sys
    import types

    try:
        from antenv.axon_hooks import get_axon_ntff_profile_hook  # noqa: F401
        return
    except ImportError:
        pass
    try:
        from trn_agent_boot.trn_boot import _ntff_profile_via_ctypes

        hook = _ntff_profile_via_ctypes("/opt/axon/libaxon_pjrt.so")
    except Exception:
        hook = None
    mod = types.ModuleType("antenv.axon_hooks")
    state = {"hook": hook}
    mod.get_axon_ntff_profile_hook = lambda: state["hook"]
    mod.set_axon_ntff_profile_hook = lambda h: state.update(hook=h)
    sys.modules["antenv.axon_hooks"] = mod


def run(x, C, U, R, trace=False, **spmd_kwargs):
    from concourse.bass_utils import run_bass_kernel_spmd

    _ensure_ntff_hook()
    nc = _build_nc()
    in_maps = _shard_inputs(x, C, U, R)
    res = run_bass_kernel_spmd(
        nc, in_maps, core_ids=list(range(NCORES)), trace=trace, **spmd_kwargs
    )
    out = _unshard_output([r["out"] for r in res.results])
    return out, res


def kernel(x, C, U, R):
    out, _ = run(x, C, U, R, trace=False)
    return out



# revision 2
# speedup vs baseline: 1.2680x; 1.2680x over previous
"""Trainium2 Bass kernel for CURLoRA forward: out = x @ (C @ U @ R).T

Fused low-rank chain per core (never materializes the [8192, 8192] W):
  t1.T = sum_k R_k.T.T @ x_k.T     (64 K-tiles of 128, bf16, PSUM-accumulated)
  t2.T = [U.T|U.T].T @ t1.T        (one bf16 matmul, M=128 duplicates t2.T
                                    into both partition halves)
  out  = t2.T.T @ C.T              (bf16 single-pass via PE quadrants)

All matmul inputs are HOST-CAST to bf16 (x, R, C, U): halves every input's
HBM bytes vs f32 and makes stage 1 single-pass on the PE (fp32 HIGH mode is
4-pass). End-to-end rel err ~4e-3 (stage-1 contraction over 8192 in bf16
contributes ~1.6e-3; stage-3 bf16 ~2.4e-3). Output stays f32.

Sharding (8 cores, no collectives): the 128 rows of x are split 4 ways and
the 8192 output columns 2 ways. Per core DMA: 0.5MB x-shard + 1MB R
(replicated; irreducible without cross-core comms) + 0.5MB C.T shard +
16KB U + 0.5MB out f32 write = ~2.5MB vs 4.7MB for the f32 version.

Hand-scheduled raw bass (no Tile): per-DMA semaphores, engine-parallel
descriptor generation (x on sync, R on scalar, C.T/U on gpsimd), C.T gated
behind the first x piece so the stage-1-critical bytes keep the HBM
bandwidth, and a pipelined matmul->copy->DMA output tail per PSUM bank.
Each engine's final out-DMA wait is kept (halting engines with DMAs in
flight risks wedging the device)."""

import numpy as np

B, S, M, N, RANK = 2, 64, 8192, 8192, 64
NCORES = 8
SA, NB = 4, 2              # s-blocks x n-blocks = 8 cores
SSH = (B * S) // SA        # 32 s-rows per core
NSH = N // NB              # 4096 out cols per core
KCH = M // 128             # 64 contraction chunks of 128

# k-chunks per x/R DMA piece: small first piece starts the PE early, small
# last piece keeps the post-stream PE chase short
PIECES = (8, 24, 24, 8)

_NC_CACHE = {}


def _build_nc():
    if "nc" in _NC_CACHE:
        return _NC_CACHE["nc"]
    from contextlib import ExitStack
    from concourse import mybir
    import concourse.bass as bass

    f32 = mybir.dt.float32
    bf16 = mybir.dt.bfloat16
    nc = bass.Bass()

    xp_d = nc.declare_dram_parameter("xp", [128, KCH * SSH], bf16, isOutput=False)
    rp_d = nc.declare_dram_parameter("rp", [128, KCH * RANK], bf16, isOutput=False)
    uq_d = nc.declare_dram_parameter("uq", [RANK, 128], bf16, isOutput=False)
    ct_d = nc.declare_dram_parameter("ct", [128, NSH // 2], bf16, isOutput=False)
    out_d = nc.declare_dram_parameter("out", [128, NSH // 4], f32, isOutput=True)

    ctx = ExitStack()
    with ctx:
        xts = [
            ctx.enter_context(nc.sbuf_tensor(f"xt{i}", [128, kw * SSH], bf16))
            for i, kw in enumerate(PIECES)
        ]
        rts = [
            ctx.enter_context(nc.sbuf_tensor(f"rt{i}", [128, kw * RANK], bf16))
            for i, kw in enumerate(PIECES)
        ]
        uqt = ctx.enter_context(nc.sbuf_tensor("uqt", [RANK, 128], bf16))
        cts = [
            ctx.enter_context(nc.sbuf_tensor(f"ct{i}", [128, 1024], bf16))
            for i in range(2)
        ]
        t1s = ctx.enter_context(nc.sbuf_tensor("t1s", [RANK, SSH], bf16))
        t2b = ctx.enter_context(nc.sbuf_tensor("t2b", [128, SSH], bf16))
        osbs = [
            ctx.enter_context(nc.sbuf_tensor(f"osb{i}", [128, 256], f32))
            for i in range(4)
        ]
        # one PSUM bank each ([128, 512] f32 = exactly one bank)
        ps1 = ctx.enter_context(nc.psum_tensor("ps1", [128, 512], f32))
        ps2 = ctx.enter_context(nc.psum_tensor("ps2", [128, 512], f32))
        psos = [
            ctx.enter_context(nc.psum_tensor(f"pso{i}", [128, 512], f32))
            for i in range(4)
        ]

        # one semaphore per DMA: queue completions of distinct DMAs are not
        # ordered, so a shared counter would be unsound
        sxs = [ctx.enter_context(nc.semaphore(f"sx{i}")) for i in range(len(PIECES))]
        srs = [ctx.enter_context(nc.semaphore(f"sr{i}")) for i in range(len(PIECES))]
        scu = ctx.enter_context(nc.semaphore("scu"))
        scs = [ctx.enter_context(nc.semaphore(f"sc{i}")) for i in range(2)]
        sm = ctx.enter_context(nc.semaphore("sm"))
        sv = ctx.enter_context(nc.semaphore("sv"))
        sos = [ctx.enter_context(nc.semaphore(f"so{i}")) for i in range(4)]

        block = ctx.enter_context(nc.Block())

        @block.sync
        def _(sync):
            off = 0
            for p, kw in enumerate(PIECES):
                sync.dma_start(
                    xts[p][:], xp_d[:, off * SSH:(off + kw) * SSH]
                ).then_inc(sxs[p], 16)
                off += kw
            for cb in (0, 1):
                sync.wait_ge(sv, 3 + cb)
                sync.dma_start(
                    out_d[:, cb * 256:(cb + 1) * 256], osbs[cb][:]
                ).then_inc(sos[cb], 16)
            # wait only on the LAST out DMA of this ring: per-engine FIFO
            # descriptor drain makes its sem imply the earlier DMA finished,
            # and halting engines with DMAs in flight risks wedging the
            # device (NRT_EXEC_UNIT_UNRECOVERABLE observed twice without it)
            sync.wait_ge(sos[1], 16)

        @block.scalar
        def _(scalar):
            off = 0
            for p, kw in enumerate(PIECES):
                scalar.dma_start(
                    rts[p][:], rp_d[:, off * RANK:(off + kw) * RANK]
                ).then_inc(srs[p], 16)
                off += kw
            # second half of the output on the ACT ring: each HWDGE issue
            # occupies its engine ~0.65us, and with no final waits the LAST
            # issue sets body end -- two engines halve the serialization
            for cb in (2, 3):
                scalar.wait_ge(sv, 3 + cb)
                scalar.dma_start(
                    out_d[:, cb * 256:(cb + 1) * 256], osbs[cb][:]
                ).then_inc(sos[cb], 16)
            scalar.wait_ge(sos[3], 16)

        @block.gpsimd
        def _(g):
            g.dma_start(uqt[:], uq_d[:]).then_inc(scu, 16)
            g.wait_ge(sxs[0], 16)  # let the x/R stream lead on HBM bw
            g.dma_start(cts[0][:], ct_d[:, 0:1024]).then_inc(scs[0], 16)
            g.dma_start(cts[1][:], ct_d[:, 1024:2048]).then_inc(scs[1], 16)

        @block.tensor
        def _(t):
            k = 0
            last_mm = None
            for p, kw in enumerate(PIECES):
                t.wait_ge(sxs[p], 16)
                t.wait_ge(srs[p], 16)
                for kl in range(kw):
                    last_mm = nc.tensor.matmul(
                        ps1[0:RANK, 0:SSH],
                        rts[p][:, kl * RANK:(kl + 1) * RANK],
                        xts[p][:, kl * SSH:(kl + 1) * SSH],
                        start=(k == 0), stop=(k == KCH - 1),
                    )
                    k += 1
            last_mm.then_inc(sm, 1)                      # sm=1: stage 1 done
            t.wait_ge(sv, 1)                             # t1s copied
            t.wait_ge(scu, 16)                           # uqt loaded
            nc.tensor.matmul(ps2[:, 0:SSH], uqt[:], t1s[:],
                             start=True, stop=True).then_inc(sm, 1)  # sm=2
            t.wait_ge(sv, 2)                             # t2b copied
            t.wait_ge(scs[0], 16)                        # ct loaded (bf16)
            t.wait_ge(scs[1], 16)
            for cb in range(4):                          # 256-col out block
                rh, hb = cb // 2, cb % 2
                last_mm = None
                for p in range(2):
                    for w in range(2):
                        q = p * 2 + w                    # psum partition quarter
                        c0 = w * 512 + hb * 256
                        last_mm = nc.tensor.matmul(
                            psos[cb][q * SSH:(q + 1) * SSH, 0:256],
                            t2b[rh * 64:(rh + 1) * 64, :],
                            cts[p][rh * 64:(rh + 1) * 64, c0:c0 + 256],
                            start=True, stop=True,
                            tile_position=(rh * 64, q * SSH),
                        )
                last_mm.then_inc(sm, 1)                  # sm=3..6

        @block.vector
        def _(v):
            v.wait_ge(sm, 1)
            nc.vector.tensor_copy(t1s[:], ps1[0:RANK, 0:SSH]).then_inc(sv, 1)
            v.wait_ge(sm, 2)
            nc.vector.tensor_copy(t2b[:], ps2[:, 0:SSH]).then_inc(sv, 1)
            for cb in range(4):
                v.wait_ge(sm, 3 + cb)
                nc.vector.tensor_copy(
                    osbs[cb][:], psos[cb][:, 0:256]
                ).then_inc(sv, 1)

    _NC_CACHE["nc"] = nc
    return nc


def _shard_inputs(x, C, U, R):
    import ml_dtypes

    bf16 = ml_dtypes.bfloat16
    xf = np.asarray(x, np.float32).reshape(B * S, M)
    C = np.asarray(C, np.float32)
    U = np.asarray(U, np.float32)
    R = np.asarray(R, np.float32)

    # rp[p, k*64+r] = R[r, 128k+p]
    rp = np.ascontiguousarray(
        R.reshape(RANK, KCH, 128).transpose(2, 1, 0)
    ).reshape(128, KCH * RANK).astype(bf16)
    # uq = U.T duplicated along columns: stage 2's lhsT, M=128 so t2.T lands
    # duplicated in both partition halves (stage 3 reads them as row halves)
    uq = np.ascontiguousarray(np.concatenate([U.T, U.T], axis=1)).astype(bf16)

    in_maps = []
    for c in range(NCORES):
        i, j = divmod(c, NB)
        xs = xf[i * SSH:(i + 1) * SSH, :]
        # xp[p, k*32+s] = xs[s, 128k+p]
        xp = np.ascontiguousarray(
            xs.reshape(SSH, KCH, 128).transpose(2, 1, 0)
        ).reshape(128, KCH * SSH).astype(bf16)
        # ct rows 0:64 = C.T cols [0,2048) of this n-shard, rows 64:128 =
        # cols [2048,4096) -- full 128-partition (= full-bandwidth) DMA
        cT = C[j * NSH:(j + 1) * NSH, :].T  # [64, 4096]
        ct = np.ascontiguousarray(
            np.concatenate([cT[:, :2048], cT[:, 2048:]], axis=0)
        ).astype(bf16)  # [128, 2048]
        in_maps.append({"xp": xp, "rp": rp, "uq": uq, "ct": ct})
    return in_maps


def _unshard_output(core_outs):
    full = np.empty((B * S, N), np.float32)
    for c in range(NCORES):
        i, j = divmod(c, NB)
        q = core_outs[c]  # [128, 1024]: q[32a+s, 512h+nr] = out[s, (4h+a)*512+nr]
        blk = q.reshape(4, SSH, 2, 512).transpose(1, 2, 0, 3).reshape(SSH, NSH)
        full[i * SSH:(i + 1) * SSH, j * NSH:(j + 1) * NSH] = blk
    return full.reshape(B, S, N)


def _ensure_ntff_hook():
    """bass_utils' axon trace path imports antenv.axon_hooks, which this
    container's antenv lacks. Register an equivalent module backed by the
    boot package's ctypes NTFF hook so trace=True (or BASS_TRACE=1) works."""
    import sys
    import types

    try:
        from antenv.axon_hooks import get_axon_ntff_profile_hook  # noqa: F401
        return
    except ImportError:
        pass
    try:
        from trn_agent_boot.trn_boot import _ntff_profile_via_ctypes

        hook = _ntff_profile_via_ctypes("/opt/axon/libaxon_pjrt.so")
    except Exception:
        hook = None
    mod = types.ModuleType("antenv.axon_hooks")
    state = {"hook": hook}
    mod.get_axon_ntff_profile_hook = lambda: state["hook"]
    mod.set_axon_ntff_profile_hook = lambda h: state.update(hook=h)
    sys.modules["antenv.axon_hooks"] = mod


def run(x, C, U, R, trace=False, **spmd_kwargs):
    from concourse.bass_utils import run_bass_kernel_spmd

    _ensure_ntff_hook()
    nc = _build_nc()
    in_maps = _shard_inputs(x, C, U, R)
    res = run_bass_kernel_spmd(
        nc, in_maps, core_ids=list(range(NCORES)), trace=trace, **spmd_kwargs
    )
    out = _unshard_output([r["out"] for r in res.results])
    return out, res


def kernel(x, C, U, R):
    out, _ = run(x, C, U, R, trace=False)
    return out
